# revision 13
# baseline (speedup 1.0000x reference)
"""Routed-MoE kernel for Trainium2 (8 NeuronCores).

The reference computes all-experts MLP logits for every token and then
gathers the expert chosen by `domain`.  Only the selected expert's output is
needed, so this kernel routes on the host (argsort by expert) and runs one
expert per NeuronCore over its (capacity-padded) token group:

    core e:  out = softmax(relu(Xg[e] @ W1[e] + b1[e]) @ W2[e] + b2[e])

Key layout/performance choices:
  - No on-device transposes on the main path: L1 computes H^T [F2, tok]
    with lhsT = W1 tiles (host pre-swizzled per-m so one DMA unlocks one
    m-group) and rhs = Xg^T (host-transposed gather, streamed per
    (k, token-slice) so the PE can start after ~2 MB of DMA).
  - All matmuls run in float32r (FP22 single-pass mode, 4x true-FP32
    throughput; PSUM accumulation stays FP32).  Token slices are 384 wide
    so every matmul has output free dim >= 256 (full f32r rate).
  - L2 is computed as logits^T [C, tok] (free dim = tokens >= 256), then
    PE-transposed per 128-token tile for the free-axis softmax.
  - Token slices (0,1) are interleaved inside the m-loop so PE never
    outpaces the W1 DMA stream; slice 2 runs after weights are resident.
"""

import numpy as np
from contextlib import ExitStack

import concourse.bass as bass
import concourse.bacc as bacc
import concourse.tile as tile
from concourse import mybir
from concourse.bass_utils import run_bass_kernel_spmd
from concourse.masks import make_identity

B, E, F1, F2, C = 8192, 8, 1024, 2048, 100
N_CORES = 8
P = 128
# Token-slice widths: every slice >= 256 keeps f32r matmuls at full rate;
# small first slices minimize the DMA bytes gating the first matmul.
SLICES = [(0, 256), (256, 384), (640, 512)]
CAP = 1152          # per-expert capacity (binomial mean 1024, sd ~30)
K1 = F1 // P        # 8  K-tiles for layer 1
M1 = F2 // P        # 16 M-tiles for layer 1 (= K-tiles for layer 2)

F32 = mybir.dt.float32
F32R = mybir.dt.float32r

_CACHED_NC = None


def _build_nc():
    nc = bacc.Bacc("TRN2", target_bir_lowering=False, debug=False,
                   num_devices=N_CORES)
    xT_d = nc.dram_tensor("xT", [F1, CAP], F32R, kind="ExternalInput").ap()
    # w1s host-swizzled: w1s[m*128 + p, k*128 + j] = W1[k*128 + p, m*128 + j]
    w1_d = nc.dram_tensor("w1s", [F2, F1], F32R, kind="ExternalInput").ap()
    b1_d = nc.dram_tensor("b1r", [P, M1], F32, kind="ExternalInput").ap()
    w2_d = nc.dram_tensor("w2", [F2, C], F32R, kind="ExternalInput").ap()
    b2_d = nc.dram_tensor("b2c", [C, 1], F32, kind="ExternalInput").ap()
    out_d = nc.dram_tensor("out", [CAP, C], F32, kind="ExternalOutput").ap()

    with tile.TileContext(nc) as tc, ExitStack() as ctx:
        const = ctx.enter_context(tc.tile_pool(name="const", bufs=1))
        hpool = ctx.enter_context(tc.tile_pool(name="h", bufs=1))
        ps1 = ctx.enter_context(tc.tile_pool(name="ps1", bufs=4, space="PSUM"))
        ps2 = ctx.enter_context(tc.tile_pool(name="ps2", bufs=2, space="PSUM"))
        ps3 = ctx.enter_context(tc.tile_pool(name="ps3", bufs=2, space="PSUM"))
        lpool = ctx.enter_context(tc.tile_pool(name="l2sb", bufs=2))
        spool = ctx.enter_context(tc.tile_pool(name="stats", bufs=8))
        opool = ctx.enter_context(tc.tile_pool(name="out", bufs=8))

        ident = const.tile([P, P], F32, tag="ident")
        make_identity(nc, ident[:])
        b1sb = const.tile([P, M1], F32, tag="b1")
        nc.sync.dma_start(b1sb[:], b1_d[:])

        # x slices 0/1 first: they (plus one w1 tile) gate the first matmul
        xt = {}
        for s in (0, 1):
            n0, S = SLICES[s]
            for k in range(K1):
                t = const.tile([P, S], F32R, tag=f"x_{k}_{s}", name=f"x_{k}_{s}")
                nc.sync.dma_start(t[:], xT_d[k * P:(k + 1) * P, n0:n0 + S])
                xt[k, s] = t
        # W1: one DMA per m-group (host-swizzled rows are contiguous)
        w1t = []
        for m in range(M1):
            t = const.tile([P, K1 * P], F32R, tag=f"w1_{m}", name=f"w1_{m}")
            nc.sync.dma_start(t[:], w1_d[m * P:(m + 1) * P, :])
            w1t.append(t)
        b2sb = const.tile([P, 1], F32, tag="b2")
        nc.sync.dma_start(b2sb[:C, :], b2_d[:])
        w2t = []
        for m in range(M1):
            t = const.tile([P, C], F32R, tag=f"w2_{m}", name=f"w2_{m}")
            nc.sync.dma_start(t[:], w2_d[m * P:(m + 1) * P, :])
            w2t.append(t)
        n0, S = SLICES[2]
        for k in range(K1):
            t = const.tile([P, S], F32R, tag=f"x_{k}_2", name=f"x_{k}_2")
            nc.sync.dma_start(t[:], xT_d[k * P:(k + 1) * P, n0:n0 + S])
            xt[k, 2] = t

        # H^T for the whole batch: h[p, m*CAP + n0 + t]
        h = hpool.tile([P, M1 * CAP], F32R, tag="h")

        def l1_group(m, s):
            n0, S = SLICES[s]
            ps = ps1.tile([P, S], F32, tag="ps1", name=f"ps1_{m}_{s}")
            for k in range(K1):
                nc.tensor.matmul(
                    ps[:], w1t[m][:, k * P:(k + 1) * P], xt[k, s][:],
                    start=(k == 0), stop=(k == K1 - 1))
            nc.scalar.activation(
                h[:, m * CAP + n0: m * CAP + n0 + S], ps[:],
                mybir.ActivationFunctionType.Relu, bias=b1sb[:, m:m + 1])

        def l2_softmax(s):
            # logits^T [C, S] at full f32r rate (free dim = tokens >= 256),
            # bias folded into the PSUM evict, then per-128-token
            # PE-transpose + free-axis softmax.
            n0, S = SLICES[s]
            psl = ps2.tile([C, S], F32, tag="psl", name=f"psl_{s}")
            for m in range(M1):
                nc.tensor.matmul(
                    psl[:], w2t[m][:], h[:, m * CAP + n0: m * CAP + n0 + S],
                    start=(m == 0), stop=(m == M1 - 1))
            l2sb = lpool.tile([C, S], F32, tag="l2sb", name=f"l2sb_{s}")
            nc.scalar.activation(l2sb[:], psl[:],
                                 mybir.ActivationFunctionType.Identity,
                                 bias=b2sb[:C, :])
            for j in range(S // P):
                pst = ps3.tile([P, C], F32, tag="pst", name=f"pst_{s}_{j}")
                nc.tensor.transpose(pst[:], l2sb[:, j * P:(j + 1) * P],
                                    ident[:C, :C])
                nmx = spool.tile([P, 1], F32, tag="nmx")
                nc.vector.tensor_reduce(nmx[:], pst[:],
                                        axis=mybir.AxisListType.X,
                                        op=mybir.AluOpType.max, negate=True)
                ex = opool.tile([P, C], F32, tag="ex")
                sm = spool.tile([P, 1], F32, tag="sm")
                nc.scalar.activation(ex[:], pst[:],
                                     mybir.ActivationFunctionType.Exp,
                                     bias=nmx[:], accum_out=sm[:])
                rc = spool.tile([P, 1], F32, tag="rc")
                nc.vector.reciprocal(rc[:], sm[:])
                prob = opool.tile([P, C], F32, tag="prob")
                nc.vector.tensor_scalar_mul(prob[:], ex[:], rc[:])
                row0 = n0 + j * P
                nc.sync.dma_start(out_d[row0:row0 + P, :], prob[:])

        # Slices 0/1 interleaved in the m-loop: PE consumes one w1 m-tile
        # per ~2.1us while its DMA takes ~1.9us -> no weight-stream stall.
        for m in range(M1):
            l1_group(m, 0)
            l1_group(m, 1)
        l2_softmax(0)
        l2_softmax(1)
        for m in range(M1):
            l1_group(m, 2)
        l2_softmax(2)

    nc.compile()
    return nc


def _get_nc():
    global _CACHED_NC
    if _CACHED_NC is None:
        _CACHED_NC = _build_nc()
    return _CACHED_NC


def _np_mlp_rows(x_rows, e, W1, b1, W2, b2):
    """Host fallback (exact fp32 semantics) for capacity-overflow tokens."""
    h = np.maximum(x_rows.astype(np.float32) @ W1[e] + b1[e], 0.0)
    logits = h @ W2[e] + b2[e]
    logits -= logits.max(axis=-1, keepdims=True)
    p = np.exp(logits)
    return (p / p.sum(axis=-1, keepdims=True)).astype(np.float32)


def kernel(domain, x, W1, b1, W2, b2):
    domain = np.asarray(domain).astype(np.int64)
    x = np.ascontiguousarray(np.asarray(x, dtype=np.float32))
    W1 = np.asarray(W1, dtype=np.float32)
    b1 = np.asarray(b1, dtype=np.float32)
    W2 = np.asarray(W2, dtype=np.float32)
    b2 = np.asarray(b2, dtype=np.float32)

    order = np.argsort(domain, kind="stable")
    counts = np.bincount(domain, minlength=E).astype(np.int64)
    starts = np.concatenate([[0], np.cumsum(counts)[:-1]])

    xT = x.T  # [F1, B] view
    in_maps = []
    kept_idx = []
    for e in range(E):
        n_e = int(min(counts[e], CAP))
        idx = order[starts[e]: starts[e] + n_e]
        kept_idx.append(idx)
        xg = np.zeros((F1, CAP), dtype=np.float32)
        xg[:, :n_e] = xT[:, idx]
        # w1s[m*128+p, k*128+j] = W1[e][k*128+p, m*128+j]
        w1s = np.ascontiguousarray(
            W1[e].reshape(K1, P, M1, P).transpose(2, 1, 0, 3).reshape(F2, F1))
        in_maps.append({
            "xT": xg,
            "w1s": w1s,
            "b1r": np.ascontiguousarray(b1[e].reshape(M1, P).T),
            "w2": np.ascontiguousarray(W2[e]),
            "b2c": np.ascontiguousarray(b2[e].reshape(C, 1)),
        })

    nc = _get_nc()
    res = run_bass_kernel_spmd(nc, in_maps, list(range(N_CORES)))

    out = np.empty((B, C), dtype=np.float32)
    for e in range(E):
        idx = kept_idx[e]
        out[idx] = res.results[e]["out"][: len(idx)]
        if counts[e] > CAP:  # astronomically unlikely; exact host fallback
            ov = order[starts[e] + CAP: starts[e] + counts[e]]
            out[ov] = _np_mlp_rows(x[ov], e, W1, b1, W2, b2)

    return out


# revision 17
# speedup vs baseline: 1.1185x; 1.1185x over previous
"""Routed-MoE kernel for Trainium2 (8 NeuronCores).

The reference computes all-experts MLP logits for every token and then
gathers the expert chosen by `domain`.  Only the selected expert's output is
needed, so this kernel routes on the host (argsort by expert) and runs one
expert per NeuronCore over its (capacity-padded) token group:

    core e:  out = softmax(relu(Xg[e] @ W1[e] + b1[e]) @ W2[e] + b2[e])

Key layout/performance choices:
  - No on-device transposes on the main path: L1 computes H^T [F2, tok]
    with lhsT = W1 tiles (host pre-swizzled per-m so one DMA unlocks one
    m-group) and rhs = Xg^T (host-transposed gather, streamed per
    (k, token-slice) so the PE can start after ~2 MB of DMA).
  - All matmuls run in float32r (FP22 single-pass mode, 4x true-FP32
    throughput; PSUM accumulation stays FP32).  Token slices are 384 wide
    so every matmul has output free dim >= 256 (full f32r rate).
  - L2 is computed as logits^T [C, tok] (free dim = tokens >= 256), then
    PE-transposed per 128-token tile for the free-axis softmax.
  - Token slices (0,1) are interleaved inside the m-loop so PE never
    outpaces the W1 DMA stream; slice 2 runs after weights are resident.
"""

import numpy as np
from contextlib import ExitStack

import concourse.bass as bass
import concourse.bacc as bacc
import concourse.tile as tile
from concourse import mybir
from concourse.bass_utils import run_bass_kernel_spmd
from concourse.masks import make_identity

B, E, F1, F2, C = 8192, 8, 1024, 2048, 100
N_CORES = 8
P = 128
# Token-slice widths: every slice >= 256 keeps f32r matmuls at full rate;
# small first slices minimize the DMA bytes gating the first matmul.
SLICES = [(0, 256), (256, 384), (640, 512)]
CAP = 1152          # per-expert capacity (binomial mean 1024, sd ~30)
K1 = F1 // P        # 8  K-tiles for layer 1
M1 = F2 // P        # 16 M-tiles for layer 1 (= K-tiles for layer 2)

F32 = mybir.dt.float32
F32R = mybir.dt.float32r

_CACHED_NC = None


def _build_nc():
    nc = bacc.Bacc("TRN2", target_bir_lowering=False, debug=False,
                   num_devices=N_CORES)
    xT_d = nc.dram_tensor("xT", [F1, CAP], F32R, kind="ExternalInput").ap()
    # w1s host-swizzled: w1s[m*128 + p, k*128 + j] = W1[k*128 + p, m*128 + j]
    w1_d = nc.dram_tensor("w1s", [F2, F1], F32R, kind="ExternalInput").ap()
    b1_d = nc.dram_tensor("b1r", [P, M1], F32, kind="ExternalInput").ap()
    w2_d = nc.dram_tensor("w2", [F2, C], F32R, kind="ExternalInput").ap()
    b2_d = nc.dram_tensor("b2c", [C, 1], F32, kind="ExternalInput").ap()
    out_d = nc.dram_tensor("out", [CAP, C], F32, kind="ExternalOutput").ap()

    with tile.TileContext(nc) as tc, ExitStack() as ctx:
        const = ctx.enter_context(tc.tile_pool(name="const", bufs=1))
        hpool = ctx.enter_context(tc.tile_pool(name="h", bufs=1))
        ps1 = ctx.enter_context(tc.tile_pool(name="ps1", bufs=4, space="PSUM"))
        ps2 = ctx.enter_context(tc.tile_pool(name="ps2", bufs=2, space="PSUM"))
        ps3 = ctx.enter_context(tc.tile_pool(name="ps3", bufs=2, space="PSUM"))
        lpool = ctx.enter_context(tc.tile_pool(name="l2sb", bufs=2))
        spool = ctx.enter_context(tc.tile_pool(name="stats", bufs=8))
        opool = ctx.enter_context(tc.tile_pool(name="out", bufs=4))
        ppool = ctx.enter_context(tc.tile_pool(name="prob", bufs=1))

        ident = const.tile([P, P], F32, tag="ident")
        make_identity(nc, ident[:])
        b1sb = const.tile([P, M1], F32, tag="b1")
        nc.sync.dma_start(b1sb[:], b1_d[:])

        # One DMA per x slice / per w1 m-group (the HWDGE issue slot is
        # ~0.6us per DMACopy, so merge everything that doesn't need
        # streaming granularity).  x source viewed as [p, k, t].
        xsrc = xT_d.rearrange("(k p) c -> p k c", k=K1)
        w2src = w2_d.rearrange("(m p) c -> p m c", m=M1)

        def load_x(s):
            n0, S = SLICES[s]
            t = const.tile([P, K1, S], F32R, tag=f"x_{s}", name=f"x_{s}")
            nc.sync.dma_start(t[:], xsrc[:, :, n0:n0 + S])
            return t

        xt = {0: load_x(0)}
        w1t = []
        for m in range(M1):
            t = const.tile([P, K1 * P], F32R, tag=f"w1_{m}", name=f"w1_{m}")
            nc.sync.dma_start(t[:], w1_d[m * P:(m + 1) * P, :])
            w1t.append(t)
            if m == 0:
                xt[1] = load_x(1)
        b2sb = const.tile([P, 1], F32, tag="b2")
        nc.sync.dma_start(b2sb[:C, :], b2_d[:])
        w2sb = const.tile([P, M1, C], F32R, tag="w2")
        nc.sync.dma_start(w2sb[:], w2src[:])
        xt[2] = load_x(2)

        # H^T for the whole batch: h[p, m*CAP + n0 + t]
        h = hpool.tile([P, M1 * CAP], F32R, tag="h")

        def l1_group(m, s):
            n0, S = SLICES[s]
            ps = ps1.tile([P, S], F32, tag="ps1", name=f"ps1_{m}_{s}")
            for k in range(K1):
                nc.tensor.matmul(
                    ps[:], w1t[m][:, k * P:(k + 1) * P], xt[s][:, k, :],
                    start=(k == 0), stop=(k == K1 - 1))
            nc.scalar.activation(
                h[:, m * CAP + n0: m * CAP + n0 + S], ps[:],
                mybir.ActivationFunctionType.Relu, bias=b1sb[:, m:m + 1])

        def l2_softmax(s):
            # logits^T [C, S] at full f32r rate (free dim = tokens >= 256),
            # bias folded into the per-chunk PSUM evict, then per-128-token
            # PE-transpose + free-axis softmax.
            n0, S = SLICES[s]
            psl = ps2.tile([C, S], F32, tag="psl", name=f"psl_{s}")
            for m in range(M1):
                nc.tensor.matmul(
                    psl[:], w2sb[:, m, :], h[:, m * CAP + n0: m * CAP + n0 + S],
                    start=(m == 0), stop=(m == M1 - 1))
            nj = S // P
            prob = ppool.tile([P, nj * C], F32, tag=f"prob_{s}", name=f"prob_{s}")
            for j in range(nj):
                l2sb = lpool.tile([C, P], F32, tag="l2sb", name=f"l2sb_{s}_{j}")
                nc.scalar.activation(l2sb[:], psl[:, j * P:(j + 1) * P],
                                     mybir.ActivationFunctionType.Identity,
                                     bias=b2sb[:C, :])
                pst = ps3.tile([P, C], F32, tag="pst", name=f"pst_{s}_{j}")
                nc.tensor.transpose(pst[:], l2sb[:], ident[:C, :C])
                nmx = spool.tile([P, 1], F32, tag="nmx")
                nc.vector.tensor_reduce(nmx[:], pst[:],
                                        axis=mybir.AxisListType.X,
                                        op=mybir.AluOpType.max, negate=True)
                ex = opool.tile([P, C], F32, tag="ex")
                sm = spool.tile([P, 1], F32, tag="sm")
                nc.scalar.activation(ex[:], pst[:],
                                     mybir.ActivationFunctionType.Exp,
                                     bias=nmx[:], accum_out=sm[:])
                rc = spool.tile([P, 1], F32, tag="rc")
                nc.vector.reciprocal(rc[:], sm[:])
                nc.vector.tensor_scalar_mul(prob[:, j * C:(j + 1) * C],
                                            ex[:], rc[:])
            dst = out_d[n0:n0 + S, :].rearrange("(j p) c -> p j c", j=nj)
            nc.sync.dma_start(dst, prob[:].rearrange("p (j c) -> p j c", j=nj))

        # Slices 0/1 interleaved in the m-loop: PE consumes one w1 m-tile
        # per ~2.1us while its DMA takes ~1.9us -> no weight-stream stall.
        l1_group(0, 0)
        l1_group(0, 1)
        for m in range(1, M1):
            l1_group(m, 0)
            l1_group(m, 1)
        l2_softmax(0)
        l2_softmax(1)
        for m in range(M1):
            l1_group(m, 2)
        l2_softmax(2)

    nc.compile()
    return nc


def _get_nc():
    global _CACHED_NC
    if _CACHED_NC is None:
        _CACHED_NC = _build_nc()
    return _CACHED_NC


def _np_mlp_rows(x_rows, e, W1, b1, W2, b2):
    """Host fallback (exact fp32 semantics) for capacity-overflow tokens."""
    h = np.maximum(x_rows.astype(np.float32) @ W1[e] + b1[e], 0.0)
    logits = h @ W2[e] + b2[e]
    logits -= logits.max(axis=-1, keepdims=True)
    p = np.exp(logits)
    return (p / p.sum(axis=-1, keepdims=True)).astype(np.float32)


def kernel(domain, x, W1, b1, W2, b2):
    domain = np.asarray(domain).astype(np.int64)
    x = np.ascontiguousarray(np.asarray(x, dtype=np.float32))
    W1 = np.asarray(W1, dtype=np.float32)
    b1 = np.asarray(b1, dtype=np.float32)
    W2 = np.asarray(W2, dtype=np.float32)
    b2 = np.asarray(b2, dtype=np.float32)

    order = np.argsort(domain, kind="stable")
    counts = np.bincount(domain, minlength=E).astype(np.int64)
    starts = np.concatenate([[0], np.cumsum(counts)[:-1]])

    xT = x.T  # [F1, B] view
    in_maps = []
    kept_idx = []
    for e in range(E):
        n_e = int(min(counts[e], CAP))
        idx = order[starts[e]: starts[e] + n_e]
        kept_idx.append(idx)
        xg = np.zeros((F1, CAP), dtype=np.float32)
        xg[:, :n_e] = xT[:, idx]
        # w1s[m*128+p, k*128+j] = W1[e][k*128+p, m*128+j]
        w1s = np.ascontiguousarray(
            W1[e].reshape(K1, P, M1, P).transpose(2, 1, 0, 3).reshape(F2, F1))
        in_maps.append({
            "xT": xg,
            "w1s": w1s,
            "b1r": np.ascontiguousarray(b1[e].reshape(M1, P).T),
            "w2": np.ascontiguousarray(W2[e]),
            "b2c": np.ascontiguousarray(b2[e].reshape(C, 1)),
        })

    nc = _get_nc()
    res = run_bass_kernel_spmd(nc, in_maps, list(range(N_CORES)))

    out = np.empty((B, C), dtype=np.float32)
    for e in range(E):
        idx = kept_idx[e]
        out[idx] = res.results[e]["out"][: len(idx)]
        if counts[e] > CAP:  # astronomically unlikely; exact host fallback
            ov = order[starts[e] + CAP: starts[e] + counts[e]]
            out[ov] = _np_mlp_rows(x[ov], e, W1, b1, W2, b2)

    return out


# revision 20
# speedup vs baseline: 1.1231x; 1.0041x over previous
"""Routed-MoE kernel for Trainium2 (8 NeuronCores).

The reference computes all-experts MLP logits for every token and then
gathers the expert chosen by `domain`.  Only the selected expert's output is
needed, so this kernel routes on the host (argsort by expert) and runs one
expert per NeuronCore over its (capacity-padded) token group:

    core e:  out = softmax(relu(Xg[e] @ W1[e] + b1[e]) @ W2[e] + b2[e])

Key layout/performance choices:
  - No on-device transposes on the main path: L1 computes H^T [F2, tok]
    with lhsT = W1 tiles (host pre-swizzled per-m so one DMA unlocks one
    m-group) and rhs = Xg^T (host-transposed gather, streamed per
    (k, token-slice) so the PE can start after ~2 MB of DMA).
  - All matmuls run in float32r (FP22 single-pass mode, 4x true-FP32
    throughput; PSUM accumulation stays FP32).  Token slices are 384 wide
    so every matmul has output free dim >= 256 (full f32r rate).
  - L2 is computed as logits^T [C, tok] (free dim = tokens >= 256), then
    PE-transposed per 128-token tile for the free-axis softmax.
  - Token slices (0,1) are interleaved inside the m-loop so PE never
    outpaces the W1 DMA stream; slice 2 runs after weights are resident.
"""

import numpy as np
from contextlib import ExitStack

import concourse.bass as bass
import concourse.bacc as bacc
import concourse.tile as tile
from concourse import mybir
from concourse.bass_utils import run_bass_kernel_spmd
from concourse.masks import make_identity

B, E, F1, F2, C = 8192, 8, 1024, 2048, 100
N_CORES = 8
P = 128
# Token-slice widths: every slice >= 256 keeps f32r matmuls at full rate;
# small first slices minimize the DMA bytes gating the first matmul.
SLICES = [(0, 256), (256, 384), (640, 512)]
CAP = 1152          # per-expert capacity (binomial mean 1024, sd ~30)
K1 = F1 // P        # 8  K-tiles for layer 1
M1 = F2 // P        # 16 M-tiles for layer 1 (= K-tiles for layer 2)

F32 = mybir.dt.float32
F32R = mybir.dt.float32r

_CACHED_NC = None


def _build_nc():
    nc = bacc.Bacc("TRN2", target_bir_lowering=False, debug=False,
                   num_devices=N_CORES)
    xT_d = nc.dram_tensor("xT", [F1, CAP], F32R, kind="ExternalInput").ap()
    # w1s host-swizzled: w1s[m*128 + p, k*128 + j] = W1[k*128 + p, m*128 + j]
    w1_d = nc.dram_tensor("w1s", [F2, F1], F32R, kind="ExternalInput").ap()
    b1_d = nc.dram_tensor("b1r", [P, M1], F32, kind="ExternalInput").ap()
    w2_d = nc.dram_tensor("w2", [F2, C], F32R, kind="ExternalInput").ap()
    b2_d = nc.dram_tensor("b2c", [C, 1], F32, kind="ExternalInput").ap()
    out_d = nc.dram_tensor("out", [CAP, C], F32, kind="ExternalOutput").ap()

    with tile.TileContext(nc) as tc, ExitStack() as ctx:
        const = ctx.enter_context(tc.tile_pool(name="const", bufs=1))
        hpool = ctx.enter_context(tc.tile_pool(name="h", bufs=1))
        ps1 = ctx.enter_context(tc.tile_pool(name="ps1", bufs=3, space="PSUM"))
        ps2 = ctx.enter_context(tc.tile_pool(name="ps2", bufs=2, space="PSUM"))
        ps3 = ctx.enter_context(tc.tile_pool(name="ps3", bufs=3, space="PSUM"))
        lpool = ctx.enter_context(tc.tile_pool(name="l2sb", bufs=4))
        spool = ctx.enter_context(tc.tile_pool(name="stats", bufs=8))
        opool = ctx.enter_context(tc.tile_pool(name="out", bufs=4))
        ppool = ctx.enter_context(tc.tile_pool(name="prob", bufs=1))

        ident = const.tile([P, P], F32, tag="ident")
        make_identity(nc, ident[:])
        b1sb = const.tile([P, M1], F32, tag="b1")
        nc.sync.dma_start(b1sb[:], b1_d[:])

        # One DMA per x slice / per w1 m-group (the HWDGE issue slot is
        # ~0.6us per DMACopy, so merge everything that doesn't need
        # streaming granularity).  x source viewed as [p, k, t].
        xsrc = xT_d.rearrange("(k p) c -> p k c", k=K1)
        w2src = w2_d.rearrange("(m p) c -> p m c", m=M1)

        def load_x(s):
            n0, S = SLICES[s]
            t = const.tile([P, K1, S], F32R, tag=f"x_{s}", name=f"x_{s}")
            nc.sync.dma_start(t[:], xsrc[:, :, n0:n0 + S])
            return t

        xt = {0: load_x(0)}
        w1t = []
        for m in range(M1):
            t = const.tile([P, K1 * P], F32R, tag=f"w1_{m}", name=f"w1_{m}")
            nc.sync.dma_start(t[:], w1_d[m * P:(m + 1) * P, :])
            w1t.append(t)
            if m == 0:
                xt[1] = load_x(1)
        b2sb = const.tile([P, 1], F32, tag="b2")
        nc.sync.dma_start(b2sb[:C, :], b2_d[:])
        w2sb = const.tile([P, M1, C], F32R, tag="w2")
        nc.sync.dma_start(w2sb[:], w2src[:])
        xt[2] = load_x(2)

        # H^T for the whole batch: h[p, m*CAP + n0 + t]
        h = hpool.tile([P, M1 * CAP], F32R, tag="h")

        def l1_group(m, s):
            n0, S = SLICES[s]
            ps = ps1.tile([P, S], F32, tag="ps1", name=f"ps1_{m}_{s}")
            for k in range(K1):
                nc.tensor.matmul(
                    ps[:], w1t[m][:, k * P:(k + 1) * P], xt[s][:, k, :],
                    start=(k == 0), stop=(k == K1 - 1))
            nc.scalar.activation(
                h[:, m * CAP + n0: m * CAP + n0 + S], ps[:],
                mybir.ActivationFunctionType.Relu, bias=b1sb[:, m:m + 1])

        def l2_softmax(s):
            # logits^T [C, S] at full f32r rate (free dim = tokens >= 256),
            # bias folded into the per-chunk PSUM evict, then per-128-token
            # PE-transpose + free-axis softmax.
            n0, S = SLICES[s]
            psl = ps2.tile([C, S], F32, tag="psl", name=f"psl_{s}")
            for m in range(M1):
                nc.tensor.matmul(
                    psl[:], w2sb[:, m, :], h[:, m * CAP + n0: m * CAP + n0 + S],
                    start=(m == 0), stop=(m == M1 - 1))
            nj = S // P
            for j in range(nj):
                l2sb = lpool.tile([C, P], F32, tag="l2sb", name=f"l2sb_{s}_{j}")
                nc.scalar.activation(l2sb[:], psl[:, j * P:(j + 1) * P],
                                     mybir.ActivationFunctionType.Identity,
                                     bias=b2sb[:C, :])
                pst = ps3.tile([P, C], F32, tag="pst", name=f"pst_{s}_{j}")
                nc.tensor.transpose(pst[:], l2sb[:], ident[:C, :C])
                nmx = spool.tile([P, 1], F32, tag="nmx")
                nc.vector.tensor_reduce(nmx[:], pst[:],
                                        axis=mybir.AxisListType.X,
                                        op=mybir.AluOpType.max, negate=True)
                ex = opool.tile([P, C], F32, tag="ex")
                sm = spool.tile([P, 1], F32, tag="sm")
                nc.scalar.activation(ex[:], pst[:],
                                     mybir.ActivationFunctionType.Exp,
                                     bias=nmx[:], accum_out=sm[:])
                rc = spool.tile([P, 1], F32, tag="rc")
                nc.vector.reciprocal(rc[:], sm[:])
                prob = ppool.tile([P, C], F32, tag="prob", bufs=4)
                nc.vector.tensor_scalar_mul(prob[:], ex[:], rc[:])
                row0 = n0 + j * P
                nc.sync.dma_start(out_d[row0:row0 + P, :], prob[:])

        # Slices 0/1 interleaved in the m-loop: PE consumes one w1 m-tile
        # per ~2.1us while its DMA takes ~1.9us -> no weight-stream stall.
        l1_group(0, 0)
        l1_group(0, 1)
        for m in range(1, M1):
            l1_group(m, 0)
            l1_group(m, 1)
        l2_softmax(0)
        l2_softmax(1)
        for m in range(M1):
            l1_group(m, 2)
        l2_softmax(2)

    nc.compile()
    return nc


def _get_nc():
    global _CACHED_NC
    if _CACHED_NC is None:
        _CACHED_NC = _build_nc()
    return _CACHED_NC


def _np_mlp_rows(x_rows, e, W1, b1, W2, b2):
    """Host fallback (exact fp32 semantics) for capacity-overflow tokens."""
    h = np.maximum(x_rows.astype(np.float32) @ W1[e] + b1[e], 0.0)
    logits = h @ W2[e] + b2[e]
    logits -= logits.max(axis=-1, keepdims=True)
    p = np.exp(logits)
    return (p / p.sum(axis=-1, keepdims=True)).astype(np.float32)


def kernel(domain, x, W1, b1, W2, b2):
    domain = np.asarray(domain).astype(np.int64)
    x = np.ascontiguousarray(np.asarray(x, dtype=np.float32))
    W1 = np.asarray(W1, dtype=np.float32)
    b1 = np.asarray(b1, dtype=np.float32)
    W2 = np.asarray(W2, dtype=np.float32)
    b2 = np.asarray(b2, dtype=np.float32)

    order = np.argsort(domain, kind="stable")
    counts = np.bincount(domain, minlength=E).astype(np.int64)
    starts = np.concatenate([[0], np.cumsum(counts)[:-1]])

    xT = x.T  # [F1, B] view
    in_maps = []
    kept_idx = []
    for e in range(E):
        n_e = int(min(counts[e], CAP))
        idx = order[starts[e]: starts[e] + n_e]
        kept_idx.append(idx)
        xg = np.zeros((F1, CAP), dtype=np.float32)
        xg[:, :n_e] = xT[:, idx]
        # w1s[m*128+p, k*128+j] = W1[e][k*128+p, m*128+j]
        w1s = np.ascontiguousarray(
            W1[e].reshape(K1, P, M1, P).transpose(2, 1, 0, 3).reshape(F2, F1))
        in_maps.append({
            "xT": xg,
            "w1s": w1s,
            "b1r": np.ascontiguousarray(b1[e].reshape(M1, P).T),
            "w2": np.ascontiguousarray(W2[e]),
            "b2c": np.ascontiguousarray(b2[e].reshape(C, 1)),
        })

    nc = _get_nc()
    res = run_bass_kernel_spmd(nc, in_maps, list(range(N_CORES)))

    out = np.empty((B, C), dtype=np.float32)
    for e in range(E):
        idx = kept_idx[e]
        out[idx] = res.results[e]["out"][: len(idx)]
        if counts[e] > CAP:  # astronomically unlikely; exact host fallback
            ov = order[starts[e] + CAP: starts[e] + counts[e]]
            out[ov] = _np_mlp_rows(x[ov], e, W1, b1, W2, b2)

    return out


# revision 23
# speedup vs baseline: 1.1492x; 1.0232x over previous
"""Routed-MoE kernel for Trainium2 (8 NeuronCores).

The reference computes all-experts MLP logits for every token and then
gathers the expert chosen by `domain`.  Only the selected expert's output is
needed, so this kernel routes on the host (argsort by expert) and runs one
expert per NeuronCore over its (capacity-padded) token group:

    core e:  out = softmax(relu(Xg[e] @ W1[e] + b1[e]) @ W2[e] + b2[e])

Key layout/performance choices:
  - No on-device transposes on the main path: L1 computes H^T [F2, tok]
    with lhsT = W1 tiles (host pre-swizzled per-m so one DMA unlocks one
    m-group) and rhs = Xg^T (host-transposed gather, streamed per
    (k, token-slice) so the PE can start after ~2 MB of DMA).
  - All matmuls run in float32r (FP22 single-pass mode, 4x true-FP32
    throughput; PSUM accumulation stays FP32).  Token slices are 384 wide
    so every matmul has output free dim >= 256 (full f32r rate).
  - L2 is computed as logits^T [C, tok] (free dim = tokens >= 256), then
    PE-transposed per 128-token tile for the free-axis softmax.
  - Token slices (0,1) are interleaved inside the m-loop so PE never
    outpaces the W1 DMA stream; slice 2 runs after weights are resident.
"""

import numpy as np
from contextlib import ExitStack

import concourse.bass as bass
import concourse.bacc as bacc
import concourse.tile as tile
from concourse import mybir
from concourse.bass_utils import run_bass_kernel_spmd
from concourse.masks import make_identity

B, E, F1, F2, C = 8192, 8, 1024, 2048, 100
N_CORES = 8
P = 128
# Token-slice widths: every slice >= 256 keeps f32r matmuls at full rate;
# small first slices minimize the DMA bytes gating the first matmul.
SLICES = [(0, 256), (256, 384), (640, 512)]
CAP = 1152          # per-expert capacity (binomial mean 1024, sd ~30)
K1 = F1 // P        # 8  K-tiles for layer 1
M1 = F2 // P        # 16 M-tiles for layer 1 (= K-tiles for layer 2)

F32 = mybir.dt.float32
F32R = mybir.dt.float32r

_CACHED_NC = None


def _build_nc():
    nc = bacc.Bacc("TRN2", target_bir_lowering=False, debug=False,
                   num_devices=N_CORES)
    xT_d = nc.dram_tensor("xT", [F1, CAP], F32R, kind="ExternalInput").ap()
    # w1s host-swizzled: w1s[m*128 + p, k*128 + j] = W1[k*128 + p, m*128 + j]
    w1_d = nc.dram_tensor("w1s", [F2, F1], F32R, kind="ExternalInput").ap()
    b1_d = nc.dram_tensor("b1r", [P, M1], F32, kind="ExternalInput").ap()
    w2_d = nc.dram_tensor("w2", [F2, C], F32R, kind="ExternalInput").ap()
    b2_d = nc.dram_tensor("b2c", [C, 1], F32, kind="ExternalInput").ap()
    out_d = nc.dram_tensor("out", [CAP, C], F32, kind="ExternalOutput").ap()

    with tile.TileContext(nc) as tc, ExitStack() as ctx:
        const = ctx.enter_context(tc.tile_pool(name="const", bufs=1))
        hpool = ctx.enter_context(tc.tile_pool(name="h", bufs=1))
        ps1 = ctx.enter_context(tc.tile_pool(name="ps1", bufs=2, space="PSUM"))
        ps2 = ctx.enter_context(tc.tile_pool(name="ps2", bufs=2, space="PSUM"))
        ps3 = ctx.enter_context(tc.tile_pool(name="ps3", bufs=4, space="PSUM"))
        lpool = ctx.enter_context(tc.tile_pool(name="l2sb", bufs=4))
        spool = ctx.enter_context(tc.tile_pool(name="stats", bufs=8))
        opool = ctx.enter_context(tc.tile_pool(name="out", bufs=4))
        ppool = ctx.enter_context(tc.tile_pool(name="prob", bufs=1))

        ident = const.tile([P, P], F32, tag="ident")
        make_identity(nc, ident[:])
        b1sb = const.tile([P, M1], F32, tag="b1")
        nc.sync.dma_start(b1sb[:], b1_d[:])

        # One DMA per x slice / per w1 m-group (the HWDGE issue slot is
        # ~0.6us per DMACopy, so merge everything that doesn't need
        # streaming granularity).  x source viewed as [p, k, t].
        xsrc = xT_d.rearrange("(k p) c -> p k c", k=K1)
        w2src = w2_d.rearrange("(m p) c -> p m c", m=M1)

        def load_x(s):
            n0, S = SLICES[s]
            t = const.tile([P, K1, S], F32R, tag=f"x_{s}", name=f"x_{s}")
            nc.sync.dma_start(t[:], xsrc[:, :, n0:n0 + S])
            return t

        xt = {0: load_x(0)}
        w1t = []
        for m in range(M1):
            t = const.tile([P, K1 * P], F32R, tag=f"w1_{m}", name=f"w1_{m}")
            nc.sync.dma_start(t[:], w1_d[m * P:(m + 1) * P, :])
            w1t.append(t)
            if m == 0:
                xt[1] = load_x(1)
        b2sb = const.tile([P, 1], F32, tag="b2")
        nc.sync.dma_start(b2sb[:C, :], b2_d[:])
        w2sb = const.tile([P, M1, C], F32R, tag="w2")
        nc.sync.dma_start(w2sb[:], w2src[:])
        xt[2] = load_x(2)

        # H^T for the whole batch: h[p, m*CAP + n0 + t]
        h = hpool.tile([P, M1 * CAP], F32R, tag="h")

        def l1_group(m, s):
            n0, S = SLICES[s]
            ps = ps1.tile([P, S], F32, tag="ps1", name=f"ps1_{m}_{s}")
            for k in range(K1):
                nc.tensor.matmul(
                    ps[:], w1t[m][:, k * P:(k + 1) * P], xt[s][:, k, :],
                    start=(k == 0), stop=(k == K1 - 1))
            nc.scalar.activation(
                h[:, m * CAP + n0: m * CAP + n0 + S], ps[:],
                mybir.ActivationFunctionType.Relu, bias=b1sb[:, m:m + 1])

        def l2_softmax(s):
            # logits^T [C, sub] at full f32r rate (free dim = tokens >= 256),
            # b2 added during the DVE evict, then per-128-token PE-transpose
            # + free-axis softmax.  No max-subtraction: logits are O(1) for
            # randn-scaled inputs (host guards isfinite and falls back).
            n0, S = SLICES[s]
            subs = [256, 256] if S == 512 else [S]
            sub0 = 0
            for S_sub in subs:
                psl = ps2.tile([C, S_sub], F32, tag="psl",
                               name=f"psl_{s}_{sub0}")
                hc0 = n0 + sub0
                for m in range(M1):
                    nc.tensor.matmul(
                        psl[:], w2sb[:, m, :],
                        h[:, m * CAP + hc0: m * CAP + hc0 + S_sub],
                        start=(m == 0), stop=(m == M1 - 1))
                for j in range(S_sub // P):
                    l2sb = lpool.tile([C, P], F32, tag="l2sb",
                                      name=f"l2sb_{s}_{sub0}_{j}")
                    nc.vector.tensor_scalar_add(l2sb[:],
                                                psl[:, j * P:(j + 1) * P],
                                                b2sb[:C, :])
                    pst = ps3.tile([P, C], F32, tag="pst",
                                   name=f"pst_{s}_{sub0}_{j}")
                    nc.tensor.transpose(pst[:], l2sb[:], ident[:C, :C])
                    ex = opool.tile([P, C], F32, tag="ex")
                    sm = spool.tile([P, 1], F32, tag="sm")
                    nc.scalar.activation(ex[:], pst[:],
                                         mybir.ActivationFunctionType.Exp,
                                         accum_out=sm[:])
                    rc = spool.tile([P, 1], F32, tag="rc")
                    nc.vector.reciprocal(rc[:], sm[:])
                    prob = ppool.tile([P, C], F32, tag="prob", bufs=4)
                    nc.vector.tensor_scalar_mul(prob[:], ex[:], rc[:])
                    row0 = hc0 + j * P
                    nc.sync.dma_start(out_d[row0:row0 + P, :], prob[:])
                sub0 += S_sub

        # Slices 0/1 interleaved in the m-loop: PE consumes one w1 m-tile
        # per ~2.1us while its DMA takes ~1.9us -> no weight-stream stall.
        l1_group(0, 0)
        l1_group(0, 1)
        for m in range(1, M1):
            l1_group(m, 0)
            l1_group(m, 1)
        l2_softmax(0)
        l2_softmax(1)
        for m in range(M1):
            l1_group(m, 2)
        l2_softmax(2)

    nc.compile()
    return nc


def _get_nc():
    global _CACHED_NC
    if _CACHED_NC is None:
        _CACHED_NC = _build_nc()
    return _CACHED_NC


def _np_mlp_rows(x_rows, e, W1, b1, W2, b2):
    """Host fallback (exact fp32 semantics) for capacity-overflow tokens."""
    h = np.maximum(x_rows.astype(np.float32) @ W1[e] + b1[e], 0.0)
    logits = h @ W2[e] + b2[e]
    logits -= logits.max(axis=-1, keepdims=True)
    p = np.exp(logits)
    return (p / p.sum(axis=-1, keepdims=True)).astype(np.float32)


def kernel(domain, x, W1, b1, W2, b2):
    domain = np.asarray(domain).astype(np.int64)
    x = np.ascontiguousarray(np.asarray(x, dtype=np.float32))
    W1 = np.asarray(W1, dtype=np.float32)
    b1 = np.asarray(b1, dtype=np.float32)
    W2 = np.asarray(W2, dtype=np.float32)
    b2 = np.asarray(b2, dtype=np.float32)

    order = np.argsort(domain, kind="stable")
    counts = np.bincount(domain, minlength=E).astype(np.int64)
    starts = np.concatenate([[0], np.cumsum(counts)[:-1]])

    xT = x.T  # [F1, B] view
    in_maps = []
    kept_idx = []
    for e in range(E):
        n_e = int(min(counts[e], CAP))
        idx = order[starts[e]: starts[e] + n_e]
        kept_idx.append(idx)
        xg = np.zeros((F1, CAP), dtype=np.float32)
        xg[:, :n_e] = xT[:, idx]
        # w1s[m*128+p, k*128+j] = W1[e][k*128+p, m*128+j]
        w1s = np.ascontiguousarray(
            W1[e].reshape(K1, P, M1, P).transpose(2, 1, 0, 3).reshape(F2, F1))
        in_maps.append({
            "xT": xg,
            "w1s": w1s,
            "b1r": np.ascontiguousarray(b1[e].reshape(M1, P).T),
            "w2": np.ascontiguousarray(W2[e]),
            "b2c": np.ascontiguousarray(b2[e].reshape(C, 1)),
        })

    nc = _get_nc()
    res = run_bass_kernel_spmd(nc, in_maps, list(range(N_CORES)))

    out = np.empty((B, C), dtype=np.float32)
    for e in range(E):
        idx = kept_idx[e]
        out[idx] = res.results[e]["out"][: len(idx)]
        if counts[e] > CAP:  # astronomically unlikely; exact host fallback
            ov = order[starts[e] + CAP: starts[e] + counts[e]]
            out[ov] = _np_mlp_rows(x[ov], e, W1, b1, W2, b2)

    # The device softmax skips max-centering (logits are O(1) for
    # randn-scaled inputs); guard against overflow just in case.
    bad = ~np.isfinite(out).all(axis=1)
    if bad.any():
        for i in np.nonzero(bad)[0]:
            out[i] = _np_mlp_rows(x[i:i + 1], int(domain[i]), W1, b1, W2, b2)

    return out
